# revision 16
# baseline (speedup 1.0000x reference)
"""Trainium2 Bass kernel: MeanHinAggregator (GNN message passing).

Reference computation (per batch-head element bh):
    z_r  = mean_n(x_neigh_r[bh, n, :]) @ w_neigh_r          (r = 0, 1)
    out  = relu(concat(x_self[bh] @ w_self, (z0 + z1) / 2) + b)

Strategy (pure data parallel over 8 NeuronCores, batch axis sharded):
  * Per core: B_shard=128, H=10 -> 1280 rows, processed in 10 groups of 128.
  * Neighbour tiles are DMA'd in natural layout [128 bh-part, (n f) free]
    (16 KiB contiguous per partition -> full DMA bandwidth).
  * The mean-over-neighbours reduction runs on the TensorEngine as 32
    accumulating matmuls with lhsT = x-slice, rhs = identity:
        psum[f, bh] += sum_k x[k, n*128+f] * I[k, bh]  ==  x[bh, n, f]
    i.e. each matmul transposes one neighbour slice into PSUM while the PSUM
    accumulation performs the sum over n.  This yields the neighbour sums
    directly in the [f, bh] layout the projection matmuls need as lhsT
    (the TensorEngine always contracts over the partition axis).
  * Projection: out[bh, d] = lhsT(sumT).T @ w.  The 1/(N*NR) mean scaling is
    folded into host-prescaled copies of w_neigh_*.  Bias is added with a
    K=1 matmul (lhsT = ones row, rhs = bias row) accumulating into PSUM.
  * Final ReLU on the Scalar engine (PSUM -> SBUF), then DMA out.
"""

import numpy as np

import concourse.bacc as bacc
import concourse.bass as bass
import concourse.tile as tile
from concourse import bass_utils, mybir
from concourse._compat import with_exitstack

B, H, N, F = 1024, 10, 32, 128
HALF = 128
D = 2 * HALF
NR = 2
NCORES = 8
BSH = B // NCORES        # 128 batch rows per core
BH = BSH * H             # 1280 (bh rows per core)
GROUP = 128              # bh rows per group
F32 = mybir.dt.float32


@with_exitstack
def _tile_kernel(ctx, tc, outs, ins, ngroups):
    nc = tc.nc
    xn0, xn1, xs, w_s, w0, w1, bvec, ident_d = ins
    (out_d,) = outs

    const = ctx.enter_context(tc.tile_pool(name="const", bufs=1))
    xpool = ctx.enter_context(tc.tile_pool(name="xp", bufs=4))
    spool = ctx.enter_context(tc.tile_pool(name="sp", bufs=3))
    opool = ctx.enter_context(tc.tile_pool(name="op", bufs=3))
    ppool = ctx.enter_context(tc.tile_pool(name="ps", bufs=2, space="PSUM"))
    pout = ctx.enter_context(tc.tile_pool(name="po", bufs=2, space="PSUM"))

    def issue_loads(g):
        """Issue the group's input DMAs: the two 2 MiB neighbour blocks on
        alternating HWDGE rings (SP / ACT) so their latencies overlap."""
        r = slice(g * GROUP, (g + 1) * GROUP)
        t0 = xpool.tile([128, N * F], F32, tag="t0")
        nc.sync.dma_start(t0[:], xn0[r, :])
        t1 = xpool.tile([128, N * F], F32, tag="t1")
        nc.scalar.dma_start(t1[:], xn1[r, :])
        ts = spool.tile([128, F], F32, tag="ts")
        nc.sync.dma_start(ts[:], xs[r, :])
        return t0, t1, ts

    pending = issue_loads(0)

    ident = const.tile([128, 128], F32, tag="ident")
    nc.sync.dma_start(ident[:], ident_d[:])
    wS_t = const.tile([128, HALF], F32, tag="wS")
    nc.sync.dma_start(wS_t[:], w_s[:])
    w0_t = const.tile([128, HALF], F32, tag="w0")
    nc.sync.dma_start(w0_t[:], w0[:])
    w1_t = const.tile([128, HALF], F32, tag="w1")
    nc.sync.dma_start(w1_t[:], w1[:])
    b_t = const.tile([1, D], F32, tag="b")
    nc.sync.dma_start(b_t[:], bvec[:])
    ones_t = const.tile([1, 128], F32, tag="ones")
    nc.vector.memset(ones_t[:], 1.0)

    for g in range(ngroups):
        r = slice(g * GROUP, (g + 1) * GROUP)
        t0, t1, ts = pending
        if g + 1 < ngroups:
            pending = issue_loads(g + 1)

        # Fold 32 neighbour slices down to 8 with two in-place strided adds on
        # the (otherwise idle) Vector engine: fp32 matmuls pay a double
        # LDWEIGHTS+MATMUL pass on the PE, so each fold level halves PE work.
        # After folding, t[:, k*F:(k+1)*F] (k<8) holds sums of 4 neighbours.
        NFOLD = 8
        for t in (t0, t1):
            nc.vector.tensor_add(t[:, 0:16 * F], t[:, 0:16 * F], t[:, 16 * F:32 * F])
            nc.vector.tensor_add(t[:, 0:8 * F], t[:, 0:8 * F], t[:, 8 * F:16 * F])

        # Neighbour sums, transposed: psum[f, bh] = sum_n x[bh, n, f]
        p0 = ppool.tile([128, 128], F32, tag="p0")
        for n in range(NFOLD):
            nc.tensor.matmul(
                p0[:], t0[:, n * F:(n + 1) * F], ident[:],
                start=(n == 0), stop=(n == NFOLD - 1),
            )
        p1 = ppool.tile([128, 128], F32, tag="p1")
        for n in range(NFOLD):
            nc.tensor.matmul(
                p1[:], t1[:, n * F:(n + 1) * F], ident[:],
                start=(n == 0), stop=(n == NFOLD - 1),
            )
        # Self transpose: psum[f, bh] = x_self[bh, f]
        pxs = ppool.tile([128, 128], F32, tag="pxs")
        nc.tensor.matmul(pxs[:], ts[:], ident[:], start=True, stop=True)

        # PSUM -> SBUF (matmul lhsT must live in SBUF).  Explicitly on DVE:
        # keeping compute waits off the ACT/SP sequencer FIFOs keeps the two
        # HWDGE rings free to issue the next group's loads immediately.
        s0 = spool.tile([128, 128], F32, tag="s0")
        nc.vector.tensor_copy(s0[:], p0[:])
        s1 = spool.tile([128, 128], F32, tag="s1")
        nc.vector.tensor_copy(s1[:], p1[:])
        sxs = spool.tile([128, 128], F32, tag="sxs")
        nc.vector.tensor_copy(sxs[:], pxs[:])

        # Projection: out[bh, d]
        po = pout.tile([128, D], F32, tag="po")
        nc.tensor.matmul(po[:, 0:HALF], sxs[:], wS_t[:], start=True, stop=False)
        nc.tensor.matmul(po[:, 0:HALF], ones_t[:], b_t[:, 0:HALF],
                         start=False, stop=True)
        nc.tensor.matmul(po[:, HALF:D], s0[:], w0_t[:], start=True, stop=False)
        nc.tensor.matmul(po[:, HALF:D], s1[:], w1_t[:], start=False, stop=False)
        nc.tensor.matmul(po[:, HALF:D], ones_t[:], b_t[:, HALF:D],
                         start=False, stop=True)

        ob = opool.tile([128, D], F32, tag="ob")
        nc.vector.tensor_scalar_max(ob[:], po[:], 0.0)
        nc.gpsimd.dma_start(out_d[r, :], ob[:])


def build_nc(ngroups=BH // GROUP):
    bh = ngroups * GROUP
    nc = bacc.Bacc("TRN2", target_bir_lowering=False, debug=False)
    xn0 = nc.dram_tensor("xn0", [bh, N * F], F32, kind="ExternalInput")
    xn1 = nc.dram_tensor("xn1", [bh, N * F], F32, kind="ExternalInput")
    xs = nc.dram_tensor("xs", [bh, F], F32, kind="ExternalInput")
    w_s = nc.dram_tensor("w_s", [F, HALF], F32, kind="ExternalInput")
    w0 = nc.dram_tensor("w0", [F, HALF], F32, kind="ExternalInput")
    w1 = nc.dram_tensor("w1", [F, HALF], F32, kind="ExternalInput")
    bvec = nc.dram_tensor("bvec", [1, D], F32, kind="ExternalInput")
    ident_d = nc.dram_tensor("ident", [128, 128], F32, kind="ExternalInput")
    out = nc.dram_tensor("out", [bh, D], F32, kind="ExternalOutput")

    ins = [t.ap() for t in (xn0, xn1, xs, w_s, w0, w1, bvec, ident_d)]
    with tile.TileContext(nc) as tc:
        _tile_kernel(tc, [out.ap()], ins, ngroups)
    nc.compile()
    return nc


def make_in_maps(x_self, x_neigh_0, x_neigh_1, w_self, w_neigh_0, w_neigh_1, b):
    """Shard full inputs into per-core input maps (batch axis, 8 ways)."""
    x_self = np.ascontiguousarray(np.asarray(x_self, dtype=np.float32))
    x_neigh_0 = np.ascontiguousarray(np.asarray(x_neigh_0, dtype=np.float32))
    x_neigh_1 = np.ascontiguousarray(np.asarray(x_neigh_1, dtype=np.float32))
    scale = np.float32(1.0 / (N * NR))
    w_s = np.ascontiguousarray(np.asarray(w_self, dtype=np.float32))
    w0 = np.ascontiguousarray(np.asarray(w_neigh_0, dtype=np.float32) * scale)
    w1 = np.ascontiguousarray(np.asarray(w_neigh_1, dtype=np.float32) * scale)
    bvec = np.ascontiguousarray(np.asarray(b, dtype=np.float32).reshape(1, D))
    ident = np.eye(128, dtype=np.float32)

    in_maps = []
    for c in range(NCORES):
        bs = slice(c * BSH, (c + 1) * BSH)
        in_maps.append({
            "xn0": np.ascontiguousarray(x_neigh_0[bs].reshape(BH, N * F)),
            "xn1": np.ascontiguousarray(x_neigh_1[bs].reshape(BH, N * F)),
            "xs": np.ascontiguousarray(x_self[bs].reshape(BH, F)),
            "w_s": w_s, "w0": w0, "w1": w1, "bvec": bvec, "ident": ident,
        })
    return in_maps


_NC_CACHE = None


def kernel(x_self, x_neigh_0, x_neigh_1, w_self, w_neigh_0, w_neigh_1, b):
    global _NC_CACHE
    if _NC_CACHE is None:
        _NC_CACHE = build_nc()
    in_maps = make_in_maps(x_self, x_neigh_0, x_neigh_1,
                           w_self, w_neigh_0, w_neigh_1, b)
    res = bass_utils.run_bass_kernel_spmd(
        _NC_CACHE, in_maps, core_ids=list(range(NCORES)))
    out = np.concatenate([r["out"] for r in res.results], axis=0)
    return out.reshape(B, H, D)


# revision 19
# speedup vs baseline: 1.0918x; 1.0918x over previous
"""Trainium2 Bass kernel: MeanHinAggregator (GNN message passing).

Reference computation (per batch-head element bh):
    z_r  = mean_n(x_neigh_r[bh, n, :]) @ w_neigh_r          (r = 0, 1)
    out  = relu(concat(x_self[bh] @ w_self, (z0 + z1) / 2) + b)

Strategy (pure data parallel over 8 NeuronCores, batch axis sharded):
  * Per core: B_shard=128, H=10 -> 1280 rows, processed in 10 groups of 128.
  * Neighbour tiles are DMA'd in natural layout [128 bh-part, (n f) free]
    (16 KiB contiguous per partition -> full DMA bandwidth).
  * The mean-over-neighbours reduction is split between engines: two in-place
    strided adds on the Vector engine fold the 32 neighbour slices to 8 (fp32
    matmuls cost a double LDWEIGHTS+MATMUL pass on TRN2, so DVE folding is
    ~4x cheaper per element than PE matmuls), then 8 accumulating matmuls
    with lhsT = x-slice, rhs = identity finish the sum:
        psum[f, bh] += sum_k x[k, n*128+f] * I[k, bh]  ==  x[bh, n, f]
    i.e. each matmul transposes one folded slice into PSUM while the PSUM
    accumulation performs the remaining sum over n.  This yields the
    neighbour sums directly in the [f, bh] layout the projection matmuls
    need as lhsT (the TensorEngine always contracts over the partition axis).
  * Projection: out[bh, d] = lhsT(sumT).T @ w.  The 1/(N*NR) mean scaling is
    folded into host-prescaled copies of w_neigh_*.  Bias is added with a
    K=1 matmul (lhsT = ones row, rhs = bias row) accumulating into PSUM.
  * Final ReLU on the Scalar engine (PSUM -> SBUF), then DMA out.
"""

import numpy as np

import concourse.bacc as bacc
import concourse.bass as bass
import concourse.tile as tile
from concourse import bass_utils, mybir
from concourse._compat import with_exitstack

B, H, N, F = 1024, 10, 32, 128
HALF = 128
D = 2 * HALF
NR = 2
NCORES = 8
BSH = B // NCORES        # 128 batch rows per core
BH = BSH * H             # 1280 (bh rows per core)
GROUP = 128              # bh rows per group
F32 = mybir.dt.float32


@with_exitstack
def _tile_kernel(ctx, tc, outs, ins, ngroups):
    nc = tc.nc
    xn0, xn1, xs, w_s, w0, w1, bvec, ident_d = ins
    (out_d,) = outs

    const = ctx.enter_context(tc.tile_pool(name="const", bufs=1))
    xpool = ctx.enter_context(tc.tile_pool(name="xp", bufs=4))
    spool = ctx.enter_context(tc.tile_pool(name="sp", bufs=3))
    opool = ctx.enter_context(tc.tile_pool(name="op", bufs=3))
    ppool = ctx.enter_context(tc.tile_pool(name="ps", bufs=2, space="PSUM"))
    pout = ctx.enter_context(tc.tile_pool(name="po", bufs=2, space="PSUM"))

    def issue_loads(g):
        """Issue the group's input DMAs: the two 2 MiB neighbour blocks on
        alternating HWDGE rings (SP / ACT) so their latencies overlap."""
        r = slice(g * GROUP, (g + 1) * GROUP)
        t0 = xpool.tile([128, N * F], F32, tag="t0")
        nc.sync.dma_start(t0[:], xn0[r, :])
        t1 = xpool.tile([128, N * F], F32, tag="t1")
        nc.scalar.dma_start(t1[:], xn1[r, :])
        ts = spool.tile([128, F], F32, tag="ts")
        nc.sync.dma_start(ts[:], xs[r, :])
        return t0, t1, ts

    pending = issue_loads(0)

    ident = const.tile([128, 128], F32, tag="ident")
    nc.sync.dma_start(ident[:], ident_d[:])
    wS_t = const.tile([128, HALF], F32, tag="wS")
    nc.sync.dma_start(wS_t[:], w_s[:])
    w0_t = const.tile([128, HALF], F32, tag="w0")
    nc.sync.dma_start(w0_t[:], w0[:])
    w1_t = const.tile([128, HALF], F32, tag="w1")
    nc.sync.dma_start(w1_t[:], w1[:])
    b_t = const.tile([1, D], F32, tag="b")
    nc.sync.dma_start(b_t[:], bvec[:])
    ones_t = const.tile([1, 128], F32, tag="ones")
    nc.vector.memset(ones_t[:], 1.0)

    for g in range(ngroups):
        r = slice(g * GROUP, (g + 1) * GROUP)
        t0, t1, ts = pending
        if g + 1 < ngroups:
            pending = issue_loads(g + 1)

        # Fold 32 neighbour slices down to 8 with two in-place strided adds on
        # the (otherwise idle) Vector engine: fp32 matmuls pay a double
        # LDWEIGHTS+MATMUL pass on the PE, so each fold level halves PE work.
        # After folding, t[:, k*F:(k+1)*F] (k<8) holds sums of 4 neighbours.
        NFOLD = 8
        for t in (t0, t1):
            nc.vector.tensor_add(t[:, 0:16 * F], t[:, 0:16 * F], t[:, 16 * F:32 * F])
            nc.vector.tensor_add(t[:, 0:8 * F], t[:, 0:8 * F], t[:, 8 * F:16 * F])

        # Neighbour sums, transposed: psum[f, bh] = sum_n x[bh, n, f]
        p0 = ppool.tile([128, 128], F32, tag="p0")
        for n in range(NFOLD):
            nc.tensor.matmul(
                p0[:], t0[:, n * F:(n + 1) * F], ident[:],
                start=(n == 0), stop=(n == NFOLD - 1),
            )
        p1 = ppool.tile([128, 128], F32, tag="p1")
        for n in range(NFOLD):
            nc.tensor.matmul(
                p1[:], t1[:, n * F:(n + 1) * F], ident[:],
                start=(n == 0), stop=(n == NFOLD - 1),
            )
        # Self transpose: psum[f, bh] = x_self[bh, f]
        pxs = ppool.tile([128, 128], F32, tag="pxs")
        nc.tensor.matmul(pxs[:], ts[:], ident[:], start=True, stop=True)

        # PSUM -> SBUF (matmul lhsT must live in SBUF)
        s0 = spool.tile([128, 128], F32, tag="s0")
        nc.any.tensor_copy(s0[:], p0[:])
        s1 = spool.tile([128, 128], F32, tag="s1")
        nc.any.tensor_copy(s1[:], p1[:])
        sxs = spool.tile([128, 128], F32, tag="sxs")
        nc.any.tensor_copy(sxs[:], pxs[:])

        # Projection: out[bh, d]
        po = pout.tile([128, D], F32, tag="po")
        nc.tensor.matmul(po[:, 0:HALF], sxs[:], wS_t[:], start=True, stop=False)
        nc.tensor.matmul(po[:, 0:HALF], ones_t[:], b_t[:, 0:HALF],
                         start=False, stop=True)
        nc.tensor.matmul(po[:, HALF:D], s0[:], w0_t[:], start=True, stop=False)
        nc.tensor.matmul(po[:, HALF:D], s1[:], w1_t[:], start=False, stop=False)
        nc.tensor.matmul(po[:, HALF:D], ones_t[:], b_t[:, HALF:D],
                         start=False, stop=True)

        ob = opool.tile([128, D], F32, tag="ob")
        nc.scalar.activation(ob[:], po[:], mybir.ActivationFunctionType.Relu)
        nc.sync.dma_start(out_d[r, :], ob[:])


def build_nc(ngroups=BH // GROUP):
    bh = ngroups * GROUP
    nc = bacc.Bacc("TRN2", target_bir_lowering=False, debug=False)
    xn0 = nc.dram_tensor("xn0", [bh, N * F], F32, kind="ExternalInput")
    xn1 = nc.dram_tensor("xn1", [bh, N * F], F32, kind="ExternalInput")
    xs = nc.dram_tensor("xs", [bh, F], F32, kind="ExternalInput")
    w_s = nc.dram_tensor("w_s", [F, HALF], F32, kind="ExternalInput")
    w0 = nc.dram_tensor("w0", [F, HALF], F32, kind="ExternalInput")
    w1 = nc.dram_tensor("w1", [F, HALF], F32, kind="ExternalInput")
    bvec = nc.dram_tensor("bvec", [1, D], F32, kind="ExternalInput")
    ident_d = nc.dram_tensor("ident", [128, 128], F32, kind="ExternalInput")
    out = nc.dram_tensor("out", [bh, D], F32, kind="ExternalOutput")

    ins = [t.ap() for t in (xn0, xn1, xs, w_s, w0, w1, bvec, ident_d)]
    with tile.TileContext(nc) as tc:
        _tile_kernel(tc, [out.ap()], ins, ngroups)
    nc.compile()
    return nc


def make_in_maps(x_self, x_neigh_0, x_neigh_1, w_self, w_neigh_0, w_neigh_1, b):
    """Shard full inputs into per-core input maps (batch axis, 8 ways)."""
    x_self = np.ascontiguousarray(np.asarray(x_self, dtype=np.float32))
    x_neigh_0 = np.ascontiguousarray(np.asarray(x_neigh_0, dtype=np.float32))
    x_neigh_1 = np.ascontiguousarray(np.asarray(x_neigh_1, dtype=np.float32))
    scale = np.float32(1.0 / (N * NR))
    w_s = np.ascontiguousarray(np.asarray(w_self, dtype=np.float32))
    w0 = np.ascontiguousarray(np.asarray(w_neigh_0, dtype=np.float32) * scale)
    w1 = np.ascontiguousarray(np.asarray(w_neigh_1, dtype=np.float32) * scale)
    bvec = np.ascontiguousarray(np.asarray(b, dtype=np.float32).reshape(1, D))
    ident = np.eye(128, dtype=np.float32)

    in_maps = []
    for c in range(NCORES):
        bs = slice(c * BSH, (c + 1) * BSH)
        in_maps.append({
            "xn0": np.ascontiguousarray(x_neigh_0[bs].reshape(BH, N * F)),
            "xn1": np.ascontiguousarray(x_neigh_1[bs].reshape(BH, N * F)),
            "xs": np.ascontiguousarray(x_self[bs].reshape(BH, F)),
            "w_s": w_s, "w0": w0, "w1": w1, "bvec": bvec, "ident": ident,
        })
    return in_maps


_NC_CACHE = None


def kernel(x_self, x_neigh_0, x_neigh_1, w_self, w_neigh_0, w_neigh_1, b):
    global _NC_CACHE
    if _NC_CACHE is None:
        _NC_CACHE = build_nc()
    in_maps = make_in_maps(x_self, x_neigh_0, x_neigh_1,
                           w_self, w_neigh_0, w_neigh_1, b)
    res = bass_utils.run_bass_kernel_spmd(
        _NC_CACHE, in_maps, core_ids=list(range(NCORES)))
    out = np.concatenate([r["out"] for r in res.results], axis=0)
    return out.reshape(B, H, D)
